# revision 1
# baseline (speedup 1.0000x reference)
"""Multi-head causal attention (B=2, S=2048, D=1024, H=16, dh=64) on 8 TRN2 cores.

Strategy
--------
- Shard the 32 (batch, head) pairs across 8 cores, 4 pairs each (cores 0-3: b=0,
  cores 4-7: b=1). Pure data parallel, no collectives.
- Per head, compute S^T = K @ Q^T directly on the PE (contraction over dh=64 on
  the partition axis), so softmax-exp output P^T = exp(S^T) is already in the
  [k, q] layout the P@V matmul needs as lhsT/rhs -- no on-device transposes.
- Softmax without max-subtraction (scores are O(1) after the 1/sqrt(dh) scale,
  exp never overflows in fp32; identical result up to fp rounding).
- Row sums l_q come for free from the P@V matmul by appending a ones-column to
  V ([2048, 65]); output row 64 of O^T accumulates sum_k P^T[k, q].
- Normalization (divide by l) and the final [65, S] -> [S, 64] transpose happen
  on the host, so the device writes O^T straight from PSUM.
- Two heads are packed per 128 SBUF partitions; their K=64-contraction S^T
  matmuls issue to disjoint PE row-groups (tile_position auto-derived from the
  base partition) and run concurrently on the systolic array.
- The mask is handled by host-side block planning at [128 k x 512 q]
  granularity: all-masked blocks are skipped, fully-kept blocks run unmasked,
  and mixed blocks get a 0/1 multiply from a small set of deduplicated mask
  tiles uploaded per core. For the causal mask this is exactly flash-style
  block skipping (~2x work saving) with a single unique diagonal tile.
- Every partial-width block (W < 512, i.e. the diagonal) fuses both heads'
  S^T matmuls into 128-contraction matmuls over a host-built zero-interleaved
  Q tensor (qz): head A's slice in rows 0-63 / cols [0:W], head B's in rows
  64-127 / cols [W:2W], zeros elsewhere. One matmul per spanned PSUM bank
  (single PE writer per bank -- HW-safe) lands both halves contiguously, so a
  single exp instruction covers them, cutting ACT instruction count ~25%.
- All matmuls use float32r: measured on HW at fp32-level accuracy (rel err
  1.6e-4 vs f64, identical to the fp32 path) at 4x the fp32 matmul rate.
"""

import os
import sys
from contextlib import ExitStack

import numpy as np

for _p in ("/opt/trn_rl_repo", "/root/.axon_site/_ro/trn_rl_repo"):
    if os.path.isdir(_p) and _p not in sys.path:
        sys.path.insert(0, _p)
        break

import concourse.bacc as bacc  # noqa: E402
import concourse.mybir as mybir  # noqa: E402
import concourse.tile as tile  # noqa: E402
from concourse.bass_utils import run_bass_kernel_spmd  # noqa: E402

F32 = mybir.dt.float32
F32R = mybir.dt.float32r
EXP = mybir.ActivationFunctionType.Exp

N_CORES = 8
H = 16
DH = 64
QBLK = 512
KBLK = 128

# persistent-SBUF budget for mask tiles; beyond this they stream from DRAM
MASK_SBUF_LIMIT = 64 * 1024  # bytes per partition

LAST_RESULTS = None  # BassKernelResults of the most recent kernel() call


def _plan_blocks(mask):
    """Classify [KBLK x QBLK] blocks of S^T per q-chunk, union over batch.

    Returns (plans, uniq_contents):
      plans[qc] = list of (kk, c0, c1, m0, m1, uid); block covers k rows
        kk*KBLK..+KBLK and q columns qc*QBLK+c0..qc*QBLK+c1. If uid >= 0,
        multiply P^T block columns [m0, m1) by mask tile `uid`.
      uniq_contents[uid] = float32 [B, KBLK, mw] 0/1 tile (per-batch content).
    The first block of each plan covers the whole column union so its matmul
    can own start=True for the PSUM accumulation group.
    """
    B, S, _ = mask.shape
    NQ, NK = S // QBLK, S // KBLK
    uniq_keys = {}
    uniq_contents = []
    plans = []
    for qc in range(NQ):
        raw = []
        for kk in range(NK):
            sub = mask[:, qc * QBLK:(qc + 1) * QBLK, kk * KBLK:(kk + 1) * KBLK]
            anyk = sub.any(axis=(0, 2))  # [QBLK] column needed?
            if not anyk.any():
                continue
            c0 = int(anyk.argmax()) & ~3
            c1 = min(QBLK, (QBLK - int(anyk[::-1].argmax()) + 3) & ~3)
            raw.append([kk, c0, c1])
        if not raw:
            plans.append([])
            continue
        C0 = min(b[1] for b in raw)
        C1 = max(b[2] for b in raw)
        fi = next((i for i, b in enumerate(raw) if b[1] == C0 and b[2] == C1),
                  None)
        if fi is None:
            raw[0][1], raw[0][2] = C0, C1  # extend block 0 to cover the union
            fi = 0
        raw.insert(0, raw.pop(fi))
        out = []
        for kk, c0, c1 in raw:
            sub = mask[:, qc * QBLK:(qc + 1) * QBLK, kk * KBLK:(kk + 1) * KBLK]
            allk = sub.all(axis=(0, 2))
            dirty = ~allk
            dirty[:c0] = False
            dirty[c1:] = False
            if dirty.any():
                m0 = int(dirty.argmax()) & ~3
                m1 = min(QBLK, (QBLK - int(dirty[::-1].argmax()) + 3) & ~3)
                dirty[m0:m1] = True  # widened cols join the masked region
                content = np.zeros((B, KBLK, m1 - m0), np.float32)
                for bb in range(B):
                    content[bb] = sub[bb, m0:m1, :].T
                key = content.tobytes()
                uid = uniq_keys.get(key)
                if uid is None:
                    uid = len(uniq_contents)
                    uniq_keys[key] = uid
                    uniq_contents.append(content)
            else:
                m0 = m1 = 0
                uid = -1
            out.append((kk, c0, c1, m0, m1, uid))
        plans.append(out)
    mw = max((c.shape[2] for c in uniq_contents), default=1)
    uniq_padded = []
    for c in uniq_contents:
        p = np.zeros((B, KBLK, mw), np.float32)
        p[:, :, :c.shape[2]] = c
        uniq_padded.append(p)
    return plans, uniq_padded


ZW_CAP = 6144  # max fused-staging columns (24 KB/partition x 2 bufs in SBUF)


def _plan_z(plans):
    """Assign qz column offsets to fusible blocks (2W <= QBLK), per q-chunk.

    Returns (zmap, zw, qcoffs): zmap[(qc, kk)] = column offset of that
    block's [128, 2W] zero-interleaved staging slice; qcoffs[qc] = (start,
    end) column range of chunk qc's slices (for chunked loading).
    """
    zmap = {}
    zw = 0
    qcoffs = []
    for qc, blocks in enumerate(plans):
        start = zw
        for kk, c0, c1, m0, m1, uid in blocks:
            W = c1 - c0
            # W < QBLK: the two halves aren't contiguous at QBLK offsets, so
            # fusing pays. 2W > QBLK just needs one matmul per spanned bank.
            if W < QBLK and zw + 2 * W <= ZW_CAP:
                zmap[(qc, kk)] = zw
                zw += 2 * W
        qcoffs.append((start, zw))
    return zmap, zw, qcoffs


def _build(S, n_groups, n_pairs, plans, n_uniq, zinfo, mw=1, repeat=1,
           la=2, p_bufs=6, s_bufs=3, o_bufs=1, osb_bufs=4):
    """Build the single SPMD program run identically on all cores.

    repeat > 1 re-runs the whole body (for wall-clock benchmarking only).
    """
    NQ, NK = S // QBLK, S // KBLK
    VW = DH + 1  # V with ones column
    nc = bacc.Bacc("TRN2", target_bir_lowering=False, debug=False)
    qt = nc.declare_dram_parameter("qt", [n_groups, 128, S], F32R, isOutput=False)
    kt = nc.declare_dram_parameter("kt", [n_groups, 128, S], F32R, isOutput=False)
    vv = nc.declare_dram_parameter("vv", [n_pairs, 128, NK * VW], F32R,
                                   isOutput=False)
    mk = nc.declare_dram_parameter("mk", [max(n_uniq, 1), 128, mw], F32R,
                                   isOutput=False)
    zmap, zw, qcoffs = zinfo
    qz = nc.declare_dram_parameter("qz", [n_groups, 128, max(zw, 1)], F32R,
                                   isOutput=False)
    ot = nc.declare_dram_parameter("ot", [n_pairs, VW, S], F32, isOutput=True)

    with tile.TileContext(nc) as tc, ExitStack() as ctx:
        qpool = ctx.enter_context(tc.tile_pool(name="qpool", bufs=2))
        kpool = ctx.enter_context(tc.tile_pool(name="kpool", bufs=2))
        vpool = ctx.enter_context(tc.tile_pool(name="vpool", bufs=3))
        mpool = ctx.enter_context(tc.tile_pool(name="mpool", bufs=1))
        ppool = ctx.enter_context(tc.tile_pool(name="ppool", bufs=p_bufs))
        obuf = ctx.enter_context(tc.tile_pool(name="obuf", bufs=osb_bufs))
        spool = ctx.enter_context(tc.tile_pool(name="spool", bufs=s_bufs, space="PSUM"))
        opool = ctx.enter_context(tc.tile_pool(name="opool", bufs=2, space="PSUM"))

        # Trigger the ACT exp-table load at t=0 so its ~2.7us overlaps the
        # initial input DMAs instead of delaying the first real exp.
        warm = mpool.tile([128, 8], F32)
        nc.vector.memset(warm[:], 0.0)
        nc.scalar.activation(warm[:], warm[:], EXP)

        # budget the persistent-mask decision against the qz staging
        # footprint (zw cols x 4 B x 2 bufs) -- both live in SBUF for the
        # whole kernel, and together they can overflow it (HW crash, unseen
        # by the allocator) even when each alone fits
        mask_budget = max(MASK_SBUF_LIMIT - 8 * zw, 16 * 1024)
        stream_masks = max(n_uniq, 1) * mw * 4 > mask_budget
        if not stream_masks:
            mtile = mpool.tile([128, max(n_uniq, 1) * mw], F32R)

        # Zero-interleaved rhs staging tiles for fused narrow blocks: head A's
        # Q slice sits in rows 0-63 / cols [0:W], head B's in rows 64-127 /
        # cols [W:2W], zeros elsewhere (memset once; DMAs never touch the
        # zero quadrants). One 128-contraction matmul then yields both heads'
        # S^T halves contiguously in a single PSUM bank -> one exp covers
        # both. One tile per distinct W keeps stale data out.


        first_group = True
        giter = [g for _ in range(repeat) for g in range(n_groups)]
        for gi, g in enumerate(giter):
            is_last_group = gi == len(giter) - 1
            ktile = kpool.tile([128, S], F32R, tag="kt")
            qtile = qpool.tile([128, S], F32R, tag="qt")
            vtiles = [vpool.tile([128, NK * VW], F32R, tag=f"vt{h}",
                                 name=f"vt{h}") for h in range(2)]
            # chunked loads, first-needed first: the opening S-matmuls only
            # need the leading columns, so don't serialize them behind
            # monolithic 1 MB transfers (DMA is bus-serial at ~330 GB/s)
            nq4 = max(NK // 4, 1) * VW  # V quarter: one qc's worth of kk
            nc.gpsimd.dma_start(ktile[:, 0:KBLK], kt[g, :, 0:KBLK])
            if zw:
                qztile = vpool.tile([128, zw], F32R, tag="qz", name="qztile",
                                    bufs=2)
                for z0, z1 in qcoffs:
                    if z0 < z1:
                        nc.gpsimd.dma_start(qztile[:, z0:z1], qz[g, :, z0:z1])
            nc.sync.dma_start(qtile[:, 0:QBLK], qt[g, :, 0:QBLK])
            nc.sync.dma_start(ktile[:, KBLK:QBLK], kt[g, :, KBLK:QBLK])
            if first_group:
                if not stream_masks:
                    for u in range(n_uniq):
                        nc.sync.dma_start(mtile[:, u * mw:(u + 1) * mw], mk[u])
                first_group = False
            for h in range(2):
                nc.sync.dma_start(vtiles[h][:, 0:nq4], vv[2 * g + h, :, 0:nq4])
            vdone = nq4
            for c0 in range(QBLK, S, QBLK):
                nc.sync.dma_start(ktile[:, c0:c0 + QBLK], kt[g, :, c0:c0 + QBLK])
                nc.sync.dma_start(qtile[:, c0:c0 + QBLK], qt[g, :, c0:c0 + QBLK])
                v1 = min(vdone + nq4, NK * VW)
                for h in range(2):
                    if vdone < v1:
                        nc.sync.dma_start(vtiles[h][:, vdone:v1],
                                          vv[2 * g + h, :, vdone:v1])
                vdone = v1
            for h in range(2):
                if vdone < NK * VW:
                    nc.sync.dma_start(vtiles[h][:, vdone:],
                                      vv[2 * g + h, :, vdone:])

            for qc in range(NQ):
                blocks = plans[qc]
                if not blocks:
                    continue
                if is_last_group and qc == NQ - 1 and len(blocks) > 2:
                    # the kernel drain runs: last exp -> (mask mul) -> last
                    # P@V -> copy -> store. Put masked/narrow blocks early in
                    # this final chunk so the drain chain is wide & DVE-free.
                    blocks = [blocks[0]] + sorted(
                        blocks[1:], key=lambda b: (b[5] < 0, b[2] - b[1]))
                nb = len(blocks)
                o_ps = [opool.tile([VW, QBLK], F32, tag=f"o{h}", name=f"o_ps{h}",
                                   bufs=o_bufs)
                        for h in range(2)]
                LA = la  # blocks of PE-lookahead before each P@V accumulate
                staged = []
                for i in range(nb + LA):
                    if i < nb:
                        kk, c0, c1, m0, m1, uid = blocks[i]
                        W = c1 - c0
                        # NOTE: TWO matmuls writing one PSUM bank (+ an ACT
                        # read) crashes real HW. The fused path below is safe:
                        # a single matmul writes the whole [0:2W] region.
                        zoff = zmap.get((qc, kk))
                        s_ps = spool.tile([128, 2 * QBLK], F32, tag="s")
                        p_t = ppool.tile([128, 2 * QBLK], F32R, tag="p")
                        q0 = qc * QBLK + c0
                        if zoff is not None:
                            hoff = W
                            # one matmul per spanned PSUM bank (single writer
                            # per bank -- the HW-safe pattern), one exp total
                            for ci in range(0, 2 * W, QBLK):
                                ce = min(ci + QBLK, 2 * W)
                                nc.tensor.matmul(
                                    s_ps[:, ci:ce],
                                    lhsT=ktile[:, kk * KBLK:(kk + 1) * KBLK],
                                    rhs=qztile[:, zoff + ci:zoff + ce],
                                    start=True, stop=True)
                            nc.scalar.activation(p_t[:, 0:2 * W],
                                                 s_ps[:, 0:2 * W], EXP)
                        else:
                            hoff = QBLK
                            for h in range(2):
                                nc.tensor.matmul(
                                    s_ps[:, h * QBLK:h * QBLK + W],
                                    lhsT=ktile[64 * h:64 * h + 64,
                                               kk * KBLK:(kk + 1) * KBLK],
                                    rhs=qtile[64 * h:64 * h + 64, q0:q0 + W],
                                    start=True, stop=True)
                            if W == QBLK:
                                nc.scalar.activation(p_t[:, 0:2 * QBLK],
                                                     s_ps[:, 0:2 * QBLK], EXP)
                            else:
                                for h in range(2):
                                    nc.scalar.activation(
                                        p_t[:, h * QBLK:h * QBLK + W],
                                        s_ps[:, h * QBLK:h * QBLK + W], EXP)
                        if uid >= 0:
                            if stream_masks:
                                ms = mpool.tile([128, mw], F32R, tag="ms",
                                                name="ms", bufs=4)
                                nc.sync.dma_start(ms[:, 0:m1 - m0],
                                                  mk[uid, :, 0:m1 - m0])
                                mop = ms[:, 0:m1 - m0]
                            else:
                                mop = mtile[:, uid * mw:uid * mw + (m1 - m0)]
                            for h in range(2):
                                lo = h * hoff + (m0 - c0)
                                nc.vector.tensor_mul(
                                    p_t[:, lo:lo + (m1 - m0)],
                                    p_t[:, lo:lo + (m1 - m0)], mop)
                        staged.append((i, kk, c0, c1, W, hoff, p_t))
                    if i >= LA:
                        j, kk, c0, c1, W, hoff, p_t = staged[i - LA]
                        for h in range(2):
                            nc.tensor.matmul(
                                o_ps[h][:, c0:c1],
                                lhsT=vtiles[h][:, kk * VW:(kk + 1) * VW],
                                rhs=p_t[:, h * hoff:h * hoff + W],
                                start=(j == 0), stop=(j == nb - 1))
                for h in range(2):
                    dst = ot[2 * g + h, :, qc * QBLK:(qc + 1) * QBLK]
                    osb = obuf.tile([VW, QBLK], F32, tag="osb")
                    if is_last_group and qc == NQ - 1:
                        # kernel drain path: copies in parallel on DVE + ACT
                        # (ACT is idle after the final exp), stores split over
                        # three DGEs so their latencies overlap
                        hq = QBLK // 2
                        if h == 0:
                            nc.vector.tensor_copy(osb[:], o_ps[h][:])
                            nc.sync.dma_start(dst, osb[:])
                        else:
                            nc.scalar.copy(osb[:], o_ps[h][:])
                            nc.gpsimd.dma_start(dst[:, 0:hq], osb[:, 0:hq])
                            nc.scalar.dma_start(dst[:, hq:], osb[:, hq:])
                    else:
                        nc.vector.tensor_copy(osb[:], o_ps[h][:])
                        nc.gpsimd.dma_start(dst, osb[:])
    nc.finalize()
    return nc


def _make_in_maps(q4, k4, v4, maskb, uniq, n_groups, per_core, zinfo,
                  plans):
    B, S = q4.shape[0], q4.shape[1]
    NK = S // KBLK
    VW = DH + 1
    n_uniq = len(uniq)
    zmap, zw, _ = zinfo
    in_maps = []
    for c in range(N_CORES):
        qt = np.empty((n_groups, 128, S), np.float32)
        kt = np.empty((n_groups, 128, S), np.float32)
        vvv = np.empty((per_core, 128, NK * VW), np.float32)
        bs = []
        for lp in range(per_core):
            gp = c * per_core + lp
            b, h = divmod(gp, H)
            bs.append(b)
            g, half = divmod(lp, 2)
            qt[g, 64 * half:64 * half + 64] = q4[b, :, h, :].T
            kt[g, 64 * half:64 * half + 64] = k4[b, :, h, :].T
            vt = np.ones((128, NK, VW), np.float32)
            vt[:, :, :DH] = v4[b, :, h, :].reshape(NK, KBLK, DH).transpose(1, 0, 2)
            vvv[lp] = vt.reshape(128, NK * VW)
        if n_uniq:
            assert len(set(bs)) == 1, "mask tiles assume one batch per core"
            mkarr = np.ascontiguousarray(
                np.stack([uniq[u][bs[0]] for u in range(n_uniq)]))
        else:
            mkarr = np.zeros((1, 128, 1), np.float32)
        qzarr = np.zeros((n_groups, 128, max(zw, 1)), np.float32)
        for qc, blocks in enumerate(plans):
            for kk, c0, c1, m0, m1, uid in blocks:
                zoff = zmap.get((qc, kk))
                if zoff is None:
                    continue
                W = c1 - c0
                q0 = qc * QBLK + c0
                qzarr[:, 0:64, zoff:zoff + W] = qt[:, 0:64, q0:q0 + W]
                qzarr[:, 64:128, zoff + W:zoff + 2 * W] = \
                    qt[:, 64:128, q0:q0 + W]
        in_maps.append({"qt": qt, "kt": kt, "vv": vvv, "mk": mkarr,
                        "qz": qzarr})
    return in_maps


def _assemble(results, B, S, per_core):
    D = H * DH
    out = np.empty((B, S, D), np.float32)
    for c in range(N_CORES):
        otc = results[c]["ot"]  # [per_core, DH+1, S]
        for lp in range(per_core):
            gp = c * per_core + lp
            b, h = divmod(gp, H)
            l = otc[lp, DH].astype(np.float64)
            l = np.where(l == 0.0, 1.0, l)
            out[b, :, h * DH:(h + 1) * DH] = \
                (otc[lp, :DH] / l).T.astype(np.float32)
    return out


def kernel(queries, keys, values, mask):
    B, S, D = queries.shape
    assert D == H * DH
    q4 = (np.ascontiguousarray(queries, dtype=np.float32) * 0.125) \
        .reshape(B, S, H, DH)
    k4 = np.ascontiguousarray(keys, dtype=np.float32).reshape(B, S, H, DH)
    v4 = np.ascontiguousarray(values, dtype=np.float32).reshape(B, S, H, DH)
    maskb = np.asarray(mask).astype(bool)

    plans, uniq = _plan_blocks(maskb)
    zinfo = _plan_z(plans)
    per_core = (B * H) // N_CORES
    n_groups = per_core // 2

    mw = uniq[0].shape[2] if uniq else 1
    nc = _build(S, n_groups, per_core, plans, len(uniq), zinfo, mw=mw)
    in_maps = _make_in_maps(q4, k4, v4, maskb, uniq, n_groups, per_core,
                            zinfo, plans)
    try:
        res = run_bass_kernel_spmd(nc, in_maps, core_ids=list(range(N_CORES)))
    except ModuleNotFoundError:
        # BASS_TRACE set but the axon NTFF profiling hook isn't installed in
        # this container -- rerun untraced
        os.environ["BASS_NEVER_TRACE"] = "1"
        res = run_bass_kernel_spmd(nc, in_maps, core_ids=list(range(N_CORES)))
    global LAST_RESULTS
    LAST_RESULTS = res
    return _assemble(res.results, B, S, per_core)



# revision 4
# speedup vs baseline: 1.1283x; 1.1283x over previous
"""Multi-head causal attention (B=2, S=2048, D=1024, H=16, dh=64) on 8 TRN2 cores.

Strategy
--------
- Shard the 32 (batch, head) pairs across 8 cores, 4 pairs each; pure data
  parallel, no collectives. Per core: 2 groups of 2 heads packed into the 128
  SBUF partitions (64 rows each).
- All matmuls in bf16 (1 PE col/cycle at any free size). Host pre-scales Q by
  A/8 with A = 128*log2(e), so the PE writes y = A*score into PSUM -- the
  exact unit both exp paths want.
- S^T = K @ Q^T per head via two 64-contraction quadrant matmuls per
  [128k x W<=512q] block (head h uses PE rows 64h..64h+64, its own PSUM bank).
- exp is the bottleneck, so it is split across two engines:
  * ACT: exact exp (scale=1/A) writing bf16.
  * DVE: Schraudolph fast-exp -- int16(y + B) bit-cast as bf16 is
    exp(score)*(1 +- ~3%). One tensor_scalar per clean block; for diagonal
    blocks one scalar_tensor_tensor fuses the +B with a 0/1 causal-mask
    multiply (masked lanes -> int16 0 -> bf16 +0.0), so masking is free.
    The 0/1 tile is one persistent [128, 516] "triangle composite" whose
    column offset serves every diagonal block. Host-side greedy balances
    the two engines per chunk; a ~62%% exact / 38%% fast mix measures
    ~6e-3 max rel err vs the 2e-2 gate.
- P@V is flipped: out[q=128, d=65] = P^T(lhsT) @ [V | 1](rhs), 65 columns per
  (q-block, k-block) pair instead of 128 -- halves P@V PE time. PSUM
  accumulates over k-blocks; the ones-column yields the softmax denominator.
  Output rows are already [q, d], so the host only divides by column 64.
- PSUM: 2 x s_ps [128,1024] (2 banks each) + 2 x o_ps [128,2,512] (2 banks
  each) = exactly 8 banks; every bank has a single PE writer.
- PE emission interleaves each chunk's S^T blocks with slices of the previous
  chunk's P@V so the PE stays busy while ACT/DVE chew on exp.
"""

import os
import sys
from contextlib import ExitStack

import numpy as np

for _p in ("/opt/trn_rl_repo", "/root/.axon_site/_ro/trn_rl_repo"):
    if os.path.isdir(_p) and _p not in sys.path:
        sys.path.insert(0, _p)
        break

import ml_dtypes  # noqa: E402
import concourse.bacc as bacc  # noqa: E402
import concourse.mybir as mybir  # noqa: E402
import concourse.tile as tile  # noqa: E402
from concourse.bass_utils import run_bass_kernel_spmd  # noqa: E402

F32 = mybir.dt.float32
BF16 = mybir.dt.bfloat16
I16 = mybir.dt.int16
EXP = mybir.ActivationFunctionType.Exp
ALU = mybir.AluOpType
BF = ml_dtypes.bfloat16

N_CORES = 8
H = 16
DH = 64
QBLK = 512
KBLK = 128
VW = DH + 1

A_SCALE = 128.0 * np.log2(np.e)  # exp(s) == 2^(s*log2 e); bf16 bits step 1/128
B_MAGIC = 16256.0 - 5.9          # 127<<7 minus Schraudolph bias correction

# per-instruction cost estimates (ns) for the host-side engine balancer
_ACT_NS = lambda w2: w2 * 0.833 + 190.0
_DVE_STT_NS = lambda w2: w2 * 1.042 + 170.0
_DVE_MUL_NS = lambda w2: w2 * 0.26 + 170.0  # bf16 sbuf 4x mode
_COPY_NS = 305.0

LAST_RESULTS = None
LAST_NC = None
LAST_INMAPS = None


def _plan(maskb):
    """Per q-chunk block list [(kk, c0, W, dirty)], causal-verified.

    dirty blocks multiply by the triangle composite comp[:, 4:4+W] where
    comp[row, c] = 1 iff c >= row + 4.
    """
    B, S, _ = maskb.shape
    NQ, NK = S // QBLK, S // KBLK
    assert np.array_equal(maskb[0], np.tril(np.ones((S, S), bool))), \
        "kernel specialised to causal mask"
    for b in range(1, B):
        assert np.array_equal(maskb[b], maskb[0])
    plans = []
    for qc in range(NQ):
        blocks = []
        for kk in range(NK):
            c0 = kk * KBLK - qc * QBLK
            if c0 >= QBLK:
                continue  # fully masked
            c0 = max(0, c0)
            W = QBLK - c0
            dirty = kk >= 4 * qc  # leading 128 cols hold the triangle
            blocks.append((kk, c0, W, dirty))
        plans.append(blocks)
    return plans


def _assign(plans):
    """Greedy per-chunk engine assignment. Returns {(qc, kk): eng} with eng in
    {'act', 'dve', 'act+mul'}. Balances estimated ACT vs DVE ns per chunk,
    charging DVE for the previous chunk's P@V copies."""
    assign = {}
    for qc, blocks in enumerate(plans):
        act_t = 0.0
        dve_t = 4 * _COPY_NS  # prev chunk's o_ps copies land here
        # big blocks first so the tail stays balanced
        for kk, c0, W, dirty in sorted(blocks, key=lambda b: -b[2]):
            if dirty:
                cost_d = _DVE_STT_NS(2 * W)
                cost_a = _ACT_NS(2 * W)
                cost_a_mul = _DVE_MUL_NS(2 * min(W, 132))
                if dve_t + cost_d <= max(act_t + cost_a, dve_t + cost_a_mul):
                    assign[(qc, kk)] = 'dve'
                    dve_t += cost_d
                else:
                    assign[(qc, kk)] = 'act+mul'
                    act_t += cost_a
                    dve_t += cost_a_mul
            else:
                if act_t + _ACT_NS(2 * W) <= dve_t + _DVE_STT_NS(2 * W):
                    assign[(qc, kk)] = 'act'
                    act_t += _ACT_NS(2 * W)
                else:
                    assign[(qc, kk)] = 'dve'
                    dve_t += _DVE_STT_NS(2 * W)
    return assign


def _emit_order(blocks, assign, qc):
    """Interleave ACT- and DVE-assigned blocks so the two exp engines run
    concurrently through the chunk."""
    act = [b for b in blocks if assign[(qc, b[0])] != 'dve']
    dve = [b for b in blocks if assign[(qc, b[0])] == 'dve']
    out = []
    na, nd = len(act), len(dve)
    ia = id_ = 0
    for i in range(na + nd):
        # proportional merge
        if ia * max(nd, 1) <= id_ * max(na, 1):
            if ia < na:
                out.append(act[ia]); ia += 1
            else:
                out.append(dve[id_]); id_ += 1
        else:
            if id_ < nd:
                out.append(dve[id_]); id_ += 1
            else:
                out.append(act[ia]); ia += 1
    return out


def _build(S, n_groups, plans, assign):
    NQ, NK = S // QBLK, S // KBLK
    nc = bacc.Bacc("TRN2", target_bir_lowering=False, debug=False)
    qt = nc.declare_dram_parameter("qt", [n_groups, 128, S], BF16, isOutput=False)
    kt = nc.declare_dram_parameter("kt", [n_groups, 128, S], BF16, isOutput=False)
    vv = nc.declare_dram_parameter("vv", [n_groups, 128, 2, NK * VW], BF16,
                                   isOutput=False)
    cm = nc.declare_dram_parameter("cm", [128, 516], BF16, isOutput=False)
    ot = nc.declare_dram_parameter("ot", [n_groups, S, 2 * VW], F32, isOutput=True)

    with tile.TileContext(nc) as tc, ExitStack() as ctx:
        qpool = ctx.enter_context(tc.tile_pool(name="qpool", bufs=2))
        kpool = ctx.enter_context(tc.tile_pool(name="kpool", bufs=2))
        vpool = ctx.enter_context(tc.tile_pool(name="vpool", bufs=4))
        cpool = ctx.enter_context(tc.tile_pool(name="cpool", bufs=1))
        ppool = ctx.enter_context(tc.tile_pool(name="ppool", bufs=28))
        obuf = ctx.enter_context(tc.tile_pool(name="obuf", bufs=4))
        spool = ctx.enter_context(tc.tile_pool(name="spool", bufs=2, space="PSUM"))
        opool = ctx.enter_context(tc.tile_pool(name="opool", bufs=2, space="PSUM"))

        # exp-table warm-up at t=0 (~2.7us) overlaps the first input DMAs
        warm = cpool.tile([128, 8], F32)
        nc.vector.memset(warm[:], 0.0)
        nc.scalar.activation(warm[:], warm[:], EXP)

        comp = cpool.tile([128, 516], BF16)
        nc.sync.dma_start(comp[:], cm[:, :])

        # pending P@V work from the previous chunk:
        # (p_tiles, vtile, o_items) with o_items = [(qb_local, [matmuls...])]
        pending = None

        def emit_pav_slice(pend, budget):
            """Emit up to `budget` P@V matmuls from pending state."""
            done = 0
            while pend["items"] and done < budget:
                qb_j, mms, o_ps, osb, g, qb_abs = pend["items"][0]
                while mms and done < budget:
                    fn = mms.pop(0)
                    fn()
                    done += 1
                if not mms:
                    # qb complete: copy + store
                    osb3 = osb[:].rearrange("p (h w) -> p h w", h=2)
                    nc.vector.tensor_copy(osb3, o_ps[:, :, 0:VW])
                    nc.gpsimd.dma_start(
                        ot[g, qb_abs * KBLK:(qb_abs + 1) * KBLK, :], osb[:])
                    pend["items"].pop(0)
            return done

        for g in range(n_groups):
            ktile = kpool.tile([128, S], BF16, tag="kt")
            qtile = qpool.tile([128, S], BF16, tag="qt")
            vtile = vpool.tile([128, 2, NK * VW], BF16, tag="vt", name="vtile")
            # chunked loads, first-needed first
            nc.sync.dma_start(ktile[:, 0:QBLK], kt[g, :, 0:QBLK])
            nc.sync.dma_start(qtile[:, 0:QBLK], qt[g, :, 0:QBLK])
            nq4 = (NK // 4) * VW
            for h in range(2):
                nc.gpsimd.dma_start(vtile[:, h, 0:nq4], vv[g, :, h, 0:nq4])
            vdone = nq4
            for c0 in range(QBLK, S, QBLK):
                nc.sync.dma_start(ktile[:, c0:c0 + QBLK], kt[g, :, c0:c0 + QBLK])
                nc.sync.dma_start(qtile[:, c0:c0 + QBLK], qt[g, :, c0:c0 + QBLK])
                v1 = min(vdone + nq4, NK * VW)
                for h in range(2):
                    nc.gpsimd.dma_start(vtile[:, h, vdone:v1], vv[g, :, h, vdone:v1])
                vdone = v1

            for qc in range(NQ):
                blocks = _emit_order(plans[qc], assign, qc)
                nb = len(blocks)
                # total pending P@V matmuls spread over this chunk's blocks
                tot = len_pending(pending)
                per = -(-tot // nb) if nb else tot
                p_tiles = {}
                for bi, (kk, c0, W, dirty) in enumerate(blocks):
                    eng = assign[(qc, kk)]
                    s_ps = spool.tile([128, 2 * QBLK], F32, tag="s")
                    p_t = ppool.tile([128, 2 * QBLK], BF16, tag="p")
                    p_tiles[kk] = (p_t, c0, W)
                    q0 = qc * QBLK + c0
                    for h in range(2):
                        nc.tensor.matmul(
                            s_ps[:, h * QBLK:h * QBLK + W],
                            lhsT=ktile[64 * h:64 * h + 64,
                                       kk * KBLK:(kk + 1) * KBLK],
                            rhs=qtile[64 * h:64 * h + 64, q0:q0 + W],
                            start=True, stop=True)
                    if W == QBLK:
                        s_in = s_ps[:, 0:2 * QBLK]
                        p_out = p_t[:, 0:2 * QBLK]
                    else:
                        s_in = s_ps[:].rearrange(
                            "p (h w) -> p h w", h=2)[:, :, 0:W]
                        p_out = p_t[:].rearrange(
                            "p (h w) -> p h w", h=2)[:, :, 0:W]
                    if eng == 'dve':
                        if dirty:
                            m2 = comp[:, 4:4 + W].unsqueeze(1).to_broadcast(
                                [128, 2, W])
                            s3 = s_ps[:].rearrange(
                                "p (h w) -> p h w", h=2)[:, :, 0:W]
                            p3 = p_t[:].bitcast(I16).rearrange(
                                "p (h w) -> p h w", h=2)[:, :, 0:W]
                            nc.vector.scalar_tensor_tensor(
                                p3, s3, B_MAGIC, m2, ALU.add, ALU.mult)
                        else:
                            nc.vector.tensor_scalar(
                                p_t[:].bitcast(I16), s_ps[:], B_MAGIC, None,
                                ALU.add)
                    else:
                        nc.scalar.activation(p_out, s_in, EXP,
                                             scale=float(1.0 / A_SCALE))
                        if dirty:
                            md = min(W, 132)
                            m2 = comp[:, 4:4 + md].unsqueeze(1).to_broadcast(
                                [128, 2, md])
                            p3 = p_t[:].rearrange(
                                "p (h w) -> p h w", h=2)[:, :, 0:md]
                            nc.vector.tensor_mul(p3, p3, m2)
                    if pending is not None:
                        emit_pav_slice(pending, per)
                if pending is not None:
                    emit_pav_slice(pending, 1 << 30)  # drain leftovers

                # stage this chunk's P@V as pending work
                items = []
                for j in range(4):
                    qb_abs = qc * 4 + j
                    o_ps = opool.tile([128, 2, QBLK], F32, tag="o",
                                      name="o_ps")
                    osb = obuf.tile([128, 2 * VW], F32, tag="osb", name="osb")
                    mms = []
                    kks = sorted(kk for kk in p_tiles if kk <= qb_abs)
                    for ki, kk in enumerate(kks):
                        p_t, c0, W = p_tiles[kk]
                        for h in range(2):
                            def mk(kk=kk, h=h, j=j, p_t=p_t, c0=c0,
                                   o_ps=o_ps, first=(ki == 0),
                                   last=(ki == len(kks) - 1), g=g,
                                   vtile=vtile):
                                nc.tensor.matmul(
                                    o_ps[:, h, 0:VW],
                                    lhsT=p_t[:, h * QBLK + j * KBLK - c0:
                                             h * QBLK + j * KBLK - c0 + KBLK],
                                    rhs=vtile[:, h, kk * VW:kk * VW + VW],
                                    start=first, stop=last)
                            mms.append(mk)
                    items.append((j, mms, o_ps, osb, g, qb_abs))
                pending = {"items": items}
        # final drain
        emit_pav_slice(pending, 1 << 30)
    nc.finalize()
    return nc


def len_pending(pending):
    if pending is None:
        return 0
    return sum(len(m) for _, m, _, _, _, _ in pending["items"])


def _make_in_maps(q4, k4, v4, n_groups):
    B, S = q4.shape[0], q4.shape[1]
    NK = S // KBLK
    comp = (np.arange(516)[None, :] >= (np.arange(128)[:, None] + 4)
            ).astype(BF)
    in_maps = []
    for c in range(N_CORES):
        qt = np.empty((n_groups, 128, S), BF)
        kt = np.empty((n_groups, 128, S), BF)
        vvv = np.empty((n_groups, 128, 2, NK * VW), BF)
        for lp in range(2 * n_groups):
            gp = c * 2 * n_groups + lp
            b, h = divmod(gp, H)
            g, half = divmod(lp, 2)
            qt[g, 64 * half:64 * half + 64] = q4[b, :, h, :].T.astype(BF)
            kt[g, 64 * half:64 * half + 64] = k4[b, :, h, :].T.astype(BF)
            vt = np.ones((128, NK, VW), np.float32)
            vt[:, :, :DH] = v4[b, :, h, :].reshape(NK, KBLK, DH).transpose(1, 0, 2)
            vvv[g, :, half, :] = vt.reshape(128, NK * VW).astype(BF)
        in_maps.append({"qt": qt, "kt": kt, "vv": vvv, "cm": comp})
    return in_maps


def _assemble(results, B, S, n_groups):
    D = H * DH
    out = np.empty((B, S, D), np.float32)
    for c in range(N_CORES):
        otc = results[c]["ot"]  # [n_groups, S, 2*VW] f32
        for lp in range(2 * n_groups):
            gp = c * 2 * n_groups + lp
            b, h = divmod(gp, H)
            g, half = divmod(lp, 2)
            blk = otc[g, :, half * VW:(half + 1) * VW].astype(np.float64)
            l = blk[:, DH]
            l = np.where(l == 0.0, 1.0, l)
            out[b, :, h * DH:(h + 1) * DH] = \
                (blk[:, :DH] / l[:, None]).astype(np.float32)
    return out


_assign_cache = None


def kernel(queries, keys, values, mask):
    global _assign_cache, LAST_RESULTS, LAST_NC, LAST_INMAPS
    B, S, D = queries.shape
    assert D == H * DH
    qs = np.ascontiguousarray(queries, dtype=np.float32) * (A_SCALE / 8.0)
    q4 = qs.reshape(B, S, H, DH)
    k4 = np.ascontiguousarray(keys, dtype=np.float32).reshape(B, S, H, DH)
    v4 = np.ascontiguousarray(values, dtype=np.float32).reshape(B, S, H, DH)
    maskb = np.asarray(mask).astype(bool)

    plans = _plan(maskb)
    _assign_cache = _assign(plans)
    n_groups = (B * H) // N_CORES // 2

    nc = _build(S, n_groups, plans, _assign_cache)
    in_maps = _make_in_maps(q4, k4, v4, n_groups)
    try:
        res = run_bass_kernel_spmd(nc, in_maps, core_ids=list(range(N_CORES)))
    except ModuleNotFoundError:
        os.environ["BASS_NEVER_TRACE"] = "1"
        res = run_bass_kernel_spmd(nc, in_maps, core_ids=list(range(N_CORES)))
    LAST_RESULTS = res
    LAST_NC = nc
    LAST_INMAPS = in_maps
    return _assemble(res.results, B, S, n_groups)
